# revision 26
# baseline (speedup 1.0000x reference)
"""Low-rank multi-head attention (B=4, S=2048, D=1024, H=16, DK=64, R=256)
on 8 Trainium2 NeuronCores.

Sharding: core c handles batch b=c//2, query rows [qh*1024, (qh+1)*1024),
qh=c%2. All 16 heads per core; k/v projections duplicated across the two
cores of a batch (no collectives needed).

Per-core dataflow (matmul operands fp16, accumulation fp32):
  - load q/k/v slices with SWDGE cast-DMA (fp32->fp16), transpose to
    [d, s] layout on the PE (identity transpose, fp16 PSUM)
  - low-rank projections: qhT/khT [dk, s] per head-pair chunk, vh [s, hd]
    with a ones column appended for row-sum extraction
  - scores[q,k] = qhT.T @ khT per head (K=64), ACT exp with scale=1/8 and
    accum_out row sums, DVE in-place tensor_scalar normalize (fp16 4x),
    SWDGE cast-DMA (fp16->fp32) writes attn rows to HBM
  - PV needs e^T [k, q]: for N_RECOMP heads recompute scores transposed on
    PE and exp again on ACT (ACT has capacity); remaining heads use
    128x128 DMA-xbar transposes of the unnormalized e (HWDGE-limited, so
    only a few heads). ctxT[dk,q] = [vh|1].T @ eT accumulated over k;
    row 64 gives sums^T, normalized via reciprocal + PE ones-outer-product
    broadcast.
  - fc chain kept transposed: fcuT[r,q] = fc_u.T @ ctxT, then
    out[q,d] = fcuT.T @ fc_v; +residual, LayerNorm fp32, store
"""

import sys

if "/opt/trn_rl_repo" not in sys.path:
    sys.path.insert(0, "/opt/trn_rl_repo")

import numpy as np

import concourse.bass as bass
import concourse.mybir as mybir
import concourse.tile as tile
from concourse import bacc

F32 = mybir.dt.float32
F16 = mybir.dt.float16
AX = mybir.AxisListType
OP = mybir.AluOpType
ACTF = mybir.ActivationFunctionType

B, S, D, H, DK, R = 4, 2048, 1024, 16, 64, 256
Q = S // 2          # q rows per core
N_CORES = 8
N_RECOMP = 16       # heads whose e^T is recomputed on PE+ACT (rest: DMA xbar)
TEMP_INV = 1.0 / (DK ** 0.5)
LN_EPS = 1e-6

_CACHE = {}


def _build_program():
    nc = bacc.Bacc("TRN2", target_bir_lowering=False, debug=False)

    # ---- I/O ----
    q_dr = nc.dram_tensor("q_sl", [Q, D], F32, kind="ExternalInput")
    k_dr = nc.dram_tensor("k_sl", [S, D], F32, kind="ExternalInput")
    v_dr = nc.dram_tensor("v_sl", [S, D], F32, kind="ExternalInput")
    wqu_dr = nc.dram_tensor("w_qs_u", [D, R], F32, kind="ExternalInput")
    wqv_dr = nc.dram_tensor("w_qs_v", [R, H * DK], F32, kind="ExternalInput")
    wku_dr = nc.dram_tensor("w_ks_u", [D, R], F32, kind="ExternalInput")
    wkv_dr = nc.dram_tensor("w_ks_v", [R, H * DK], F32, kind="ExternalInput")
    wvu_dr = nc.dram_tensor("w_vs_u", [D, R], F32, kind="ExternalInput")
    wvv_dr = nc.dram_tensor("w_vs_v", [R, H * DK], F32, kind="ExternalInput")
    fcu_dr = nc.dram_tensor("fc_u", [H * DK, R], F32, kind="ExternalInput")
    fcv_dr = nc.dram_tensor("fc_v", [R, D], F32, kind="ExternalInput")
    g_dr = nc.dram_tensor("ln_gamma", [D], F32, kind="ExternalInput")
    bta_dr = nc.dram_tensor("ln_beta", [D], F32, kind="ExternalInput")
    attn_dr = nc.dram_tensor("attn_out", [H, Q, S], F32, kind="ExternalOutput")
    out_dr = nc.dram_tensor("out_sl", [Q, D], F32, kind="ExternalOutput")

    with tile.TileContext(nc) as tc:
        with (
            tc.tile_pool(name="rp", bufs=1) as rp,
            tc.tile_pool(name="proj", bufs=1) as prj,
            tc.tile_pool(name="psum", bufs=2, space="PSUM") as pp,
            tc.tile_pool(name="psum2", bufs=2, space="PSUM") as pp2,
            tc.tile_pool(name="psum_pv", bufs=2, space="PSUM") as pvp,
        ):
            # ---------- long-lived tensors ----------
            # qhT/khT: [(h%2)*64+dk, h//2, s] fp16
            qhT = prj.tile([128, 8, Q], F16)
            khT = prj.tile([128, 8, S], F16)
            # vh: [k%128, kc, h, dk(64)+ones(1)] fp16
            vh = prj.tile([128, 16, H, 65], F16)
            nc.vector.memset(vh[:, :, :, 64:65], 1.0)
            # ctxT: [(h%2)*64+dk, h//2, q] fp16
            ctxT = rp.tile([128, 8, Q], F16)
            ones64 = rp.tile([1, 64], F32)
            nc.vector.memset(ones64[:], 1.0)
            eps_sb = rp.tile([128, 1], F32)
            nc.vector.memset(eps_sb[:], LN_EPS)
            ident = rp.tile([128, 128], F16)
            nc.vector.memset(ident[:], 0.0)
            from concourse.masks import make_identity
            make_identity(nc, ident[:], nomemset=True)

            # ---------- phase P: projections ----------
            with (
                tc.tile_pool(name="pp_w", bufs=1) as p_w,
                tc.tile_pool(name="pp_sd", bufs=4) as p_sd,
                tc.tile_pool(name="pp_wu", bufs=2) as p_wu,
                tc.tile_pool(name="pp_T", bufs=1) as p_T,
                tc.tile_pool(name="pp_x", bufs=1) as p_x,
            ):
                wv_sb = p_w.tile([128, 2, 3, H * DK], F16)
                for i, w_dr in enumerate((wqv_dr, wkv_dr, wvv_dr)):
                    nc.gpsimd.dma_start(
                        out=wv_sb[:, :, i],
                        in_=w_dr[:].rearrange("(c p) n -> p c n", p=128))
                for ti, (x_dr, wu_dr, SB) in enumerate(
                    ((q_dr, wqu_dr, 8), (k_dr, wku_dr, 16), (v_dr, wvu_dr, 16))
                ):
                    SL = SB * 128
                    wu_sb = p_wu.tile([128, 8, R], F16, tag="wu")
                    nc.gpsimd.dma_start(
                        out=wu_sb[:],
                        in_=wu_dr[:].rearrange("(c p) r -> p c r", p=128))
                    # load [s%128, d] per s-block with cast; transpose on PE
                    x_T = p_T.tile([128, 8, S], F16, tag="T")
                    for sb in range(SB):
                        x_sd = p_sd.tile([128, D], F16, tag="sd")
                        nc.gpsimd.dma_start(
                            out=x_sd[:], in_=x_dr[sb * 128:(sb + 1) * 128, :])
                        for dg in range(2):
                            tp = pp.tile([128, 4, 128], F16, tag="mm")
                            for dj in range(4):
                                dc = dg * 4 + dj
                                nc.tensor.transpose(
                                    tp[:, dj], x_sd[:, dc * 128:(dc + 1) * 128],
                                    ident[:])
                            nc.vector.tensor_copy(
                                x_T[:, dg * 4:(dg + 1) * 4,
                                    sb * 128:(sb + 1) * 128], tp[:])
                    # u-proj -> x_u [r%128, rc, s]
                    x_u = p_x.tile([128, 2, S], F16, tag="x")
                    for rc in range(2):
                        for sn in range(SL // 512):
                            ps = pp.tile([128, 512], F32, tag="mm")
                            for dc in range(8):
                                nc.tensor.matmul(
                                    ps[:],
                                    wu_sb[:, dc, rc * 128:(rc + 1) * 128],
                                    x_T[:, dc, sn * 512:(sn + 1) * 512],
                                    start=(dc == 0), stop=(dc == 7))
                            nc.vector.tensor_copy(
                                x_u[:, rc, sn * 512:(sn + 1) * 512], ps[:])
                    if ti < 2:
                        dst = qhT if ti == 0 else khT
                        for hp in range(8):
                            for sn in range(SL // 512):
                                ps = pp.tile([128, 512], F32, tag="mm")
                                for rc in range(2):
                                    nc.tensor.matmul(
                                        ps[:],
                                        wv_sb[:, rc, ti, hp * 128:(hp + 1) * 128],
                                        x_u[:, rc, sn * 512:(sn + 1) * 512],
                                        start=(rc == 0), stop=(rc == 1))
                                nc.vector.tensor_copy(
                                    dst[:, hp, sn * 512:(sn + 1) * 512], ps[:])
                    else:
                        for kc in range(16):
                            for hn in range(2):
                                ps = pp.tile([128, 512], F32, tag="mm")
                                for rc in range(2):
                                    nc.tensor.matmul(
                                        ps[:],
                                        x_u[:, rc, kc * 128:(kc + 1) * 128],
                                        wv_sb[:, rc, 2, hn * 512:(hn + 1) * 512],
                                        start=(rc == 0), stop=(rc == 1))
                                nc.vector.tensor_copy(
                                    vh[:, kc, hn * 8:(hn + 1) * 8, 0:64], ps[:])

            # ---------- phase A: attention ----------
            with (
                tc.tile_pool(name="pa_e", bufs=3) as p_e,
                tc.tile_pool(name="pa_eT", bufs=2) as p_eT,
                tc.tile_pool(name="pa_s", bufs=6) as p_s,
            ):
                for h in range(H):
                    hp, hr = h // 2, h % 2
                    r0 = 64 * hr
                    eT = p_eT.tile([128, 16, Q], F16, tag="eT")
                    for qb in range(8):
                        qs = slice(qb * 128, (qb + 1) * 128)
                        e_sb = p_e.tile([128, S], F16, tag="e")
                        sums = p_s.tile([128, 2], F32, tag="sums")
                        for kc2 in range(2):
                            ps = pp2.tile([128, 1024], F32, tag="mm2")
                            for kn in range(2):
                                nc.tensor.matmul(
                                    ps[:, kn * 512:(kn + 1) * 512],
                                    qhT[r0:r0 + 64, hp, qs],
                                    khT[r0:r0 + 64, hp,
                                        kc2 * 1024 + kn * 512:
                                        kc2 * 1024 + (kn + 1) * 512])
                            nc.scalar.activation(
                                e_sb[:, kc2 * 1024:(kc2 + 1) * 1024], ps[:],
                                ACTF.Exp, scale=TEMP_INV,
                                accum_out=sums[:, kc2:kc2 + 1])
                        if h >= N_RECOMP:
                            # xbar path: transpose unnormalized e
                            for kc in range(16):
                                nc.sync.dma_start(
                                    out=eT[:, kc, qs],
                                    in_=e_sb[:, kc * 128:(kc + 1) * 128],
                                    transpose=True)
                        rec = p_s.tile([128, 1], F32, tag="rec")
                        nc.vector.tensor_tensor(
                            rec[:], sums[:, 0:1], sums[:, 1:2], OP.add)
                        nc.vector.reciprocal(rec[:], rec[:])
                        # normalize in place (after any transposes read e_sb)
                        nc.vector.tensor_scalar_mul(e_sb[:], e_sb[:], rec[:])
                        nc.gpsimd.dma_start(out=attn_dr[h, qs, :], in_=e_sb[:])
                    if h < N_RECOMP:
                        # recompute scores transposed + exp on ACT
                        for kc in range(16):
                            ps = pp2.tile([128, 1024], F32, tag="mm2")
                            for qn in range(2):
                                nc.tensor.matmul(
                                    ps[:, qn * 512:(qn + 1) * 512],
                                    khT[r0:r0 + 64, hp, kc * 128:(kc + 1) * 128],
                                    qhT[r0:r0 + 64, hp, qn * 512:(qn + 1) * 512])
                            nc.scalar.activation(
                                eT[:, kc, :], ps[:], ACTF.Exp, scale=TEMP_INV)
                    # PV with ones column: ctxT_unnorm + sums^T in row 64
                    for qn in range(2):
                        qsl = slice(qn * 512, (qn + 1) * 512)
                        pv = pvp.tile([65, 512], F32, tag="pv")
                        for kc in range(16):
                            nc.tensor.matmul(
                                pv[:], vh[:, kc, h, :], eT[:, kc, qsl],
                                start=(kc == 0), stop=(kc == 15))
                        # normalize ctxT by sums^T (row 64): PE ones-outer bcast
                        recT = p_s.tile([1, 512], F32, tag="recT")
                        nc.vector.reciprocal(recT[:], pv[64:65, :])
                        bc = pp.tile([64, 512], F32, tag="mm")
                        nc.tensor.matmul(bc[:], ones64[:], recT[:])
                        bc_sb = p_s.tile([64, 512], F32, tag="bc")
                        nc.vector.tensor_copy(bc_sb[:], bc[:])
                        nc.vector.tensor_tensor(
                            ctxT[r0:r0 + 64, hp, qsl], pv[0:64, :], bc_sb[:],
                            OP.mult)

            # ---------- phase F: fc + residual + LN ----------
            with (
                tc.tile_pool(name="pf", bufs=2) as pf,
                tc.tile_pool(name="pfw", bufs=1) as pfw,
            ):
                gamma_bc = pfw.tile([128, D], F32)
                beta_bc = pfw.tile([128, D], F32)
                nc.scalar.dma_start(
                    out=gamma_bc[:],
                    in_=g_dr[:].rearrange("(o d) -> o d", o=1).to_broadcast([128, D]))
                nc.scalar.dma_start(
                    out=beta_bc[:],
                    in_=bta_dr[:].rearrange("(o d) -> o d", o=1).to_broadcast([128, D]))
                fcu_sb = pfw.tile([128, 8, R], F16)
                fcv_sb = pfw.tile([128, 2, D], F16)
                nc.gpsimd.dma_start(
                    out=fcu_sb[:], in_=fcu_dr[:].rearrange("(c p) r -> p c r", p=128))
                nc.gpsimd.dma_start(
                    out=fcv_sb[:], in_=fcv_dr[:].rearrange("(c p) d -> p c d", p=128))

                fcuT = pfw.tile([128, 2, Q], F16)   # [r%128, rh, q]
                for rh in range(2):
                    for qn in range(2):
                        ps = pp.tile([128, 512], F32, tag="mm")
                        for c in range(8):
                            nc.tensor.matmul(
                                ps[:],
                                fcu_sb[:, c, rh * 128:(rh + 1) * 128],
                                ctxT[:, c, qn * 512:(qn + 1) * 512],
                                start=(c == 0), stop=(c == 7))
                        nc.vector.tensor_copy(
                            fcuT[:, rh, qn * 512:(qn + 1) * 512], ps[:])

                for qb in range(8):
                    qs = slice(qb * 128, (qb + 1) * 128)
                    ops = pp2.tile([128, 1024], F32, tag="mm2")
                    for nd in range(2):
                        for rh in range(2):
                            nc.tensor.matmul(
                                ops[:, nd * 512:(nd + 1) * 512],
                                fcuT[:, rh, qs],
                                fcv_sb[:, rh, nd * 512:(nd + 1) * 512],
                                start=(rh == 0), stop=(rh == 1))
                    res = pf.tile([128, D], F32, tag="res")
                    nc.scalar.dma_start(out=res[:], in_=q_dr[qs, :])
                    x = pf.tile([128, D], F32, tag="x")
                    nc.vector.tensor_tensor(x[:], ops[:], res[:], OP.add)
                    mean = pf.tile([128, 1], F32, tag="mean")
                    nc.vector.reduce_sum(mean[:], x[:], axis=AX.X)
                    nc.vector.tensor_scalar_mul(mean[:], mean[:], 1.0 / D)
                    xc = pf.tile([128, D], F32, tag="xc")
                    nc.vector.tensor_scalar(
                        xc[:], x[:], mean[:], None, OP.subtract)
                    sq = pf.tile([128, D], F32, tag="sq")
                    ss = pf.tile([128, 1], F32, tag="ss")
                    nc.vector.tensor_tensor(sq[:], xc[:], xc[:], OP.mult)
                    nc.vector.reduce_sum(ss[:], sq[:], axis=AX.X)
                    std = pf.tile([128, 1], F32, tag="std")
                    nc.scalar.activation(
                        std[:], ss[:], ACTF.Sqrt, scale=1.0 / D, bias=eps_sb[:])
                    rstd = pf.tile([128, 1], F32, tag="rstd")
                    nc.vector.reciprocal(rstd[:], std[:])
                    y = pf.tile([128, D], F32, tag="y")
                    nc.vector.scalar_tensor_tensor(
                        y[:], xc[:], rstd[:], gamma_bc[:], OP.mult, OP.mult)
                    yo = pf.tile([128, D], F32, tag="yo")
                    nc.vector.tensor_tensor(yo[:], y[:], beta_bc[:], OP.add)
                    nc.scalar.dma_start(out=out_dr[qs, :], in_=yo[:])

    nc.compile()
    return nc


def _get_program():
    if "nc" not in _CACHE:
        _CACHE["nc"] = _build_program()
    return _CACHE["nc"]


def kernel(q, k, v, w_qs_u, w_qs_v, w_ks_u, w_ks_v, w_vs_u, w_vs_v,
           fc_u, fc_v, ln_gamma, ln_beta):
    from concourse.bass_utils import run_bass_kernel_spmd

    f32 = lambda a: np.ascontiguousarray(np.asarray(a), dtype=np.float32)
    q, k, v = f32(q), f32(k), f32(v)
    shared = {
        "w_qs_u": f32(w_qs_u), "w_qs_v": f32(w_qs_v),
        "w_ks_u": f32(w_ks_u), "w_ks_v": f32(w_ks_v),
        "w_vs_u": f32(w_vs_u), "w_vs_v": f32(w_vs_v),
        "fc_u": f32(fc_u), "fc_v": f32(fc_v),
        "ln_gamma": f32(ln_gamma), "ln_beta": f32(ln_beta),
    }
    in_maps = []
    for c in range(N_CORES):
        b, qh = c // 2, c % 2
        m = dict(shared)
        m["q_sl"] = f32(q[b, qh * Q:(qh + 1) * Q])
        m["k_sl"] = f32(k[b])
        m["v_sl"] = f32(v[b])
        in_maps.append(m)

    nc = _get_program()
    res = run_bass_kernel_spmd(nc, in_maps, core_ids=list(range(N_CORES)))
    results = res.results

    attn = np.empty((B, H, S, S), dtype=np.float32)
    out = np.empty((B, S, D), dtype=np.float32)
    for c in range(N_CORES):
        b, qh = c // 2, c % 2
        attn[b, :, qh * Q:(qh + 1) * Q, :] = results[c]["attn_out"]
        out[b, qh * Q:(qh + 1) * Q, :] = results[c]["out_sl"]
    return out, attn
